# revision 6
# baseline (speedup 1.0000x reference)
"""BlockwiseQuantLinear Trainium2 kernel.

y = x_deq @ w_deq.T where
  x_deq = fp8-blockwise-quantize-dequantize(x)  (1x128 blocks along K)
  w_deq = fp8 weight * 128x128 blockwise scales

Strategy: data-parallel over M across the 8 NeuronCores (8192 rows each).
Weight is dequantized to bf16 on host (small: 1024x1024), transposed to
[K, N], and replicated. On device, per 128-row m-tile:
  1. DVE: segmented abs-max over 1x128 blocks -> per-(row,block) scales.
     TRN2's fp8e4 is IEEE e4m3 (max 240), not e4m3fn (max 448), so we
     quantize with 224/amax: identical RTNE rounding up to a power of two.
  2. DVE: x * (224/amax) -> xq in fp8e4, and diag matrices diag(amax/224).
  3. PE: xq_block^T @ diag -> PSUM, which both transposes the block (K on
     partitions) and applies the dequant scale in one matmul.
  4. ACT: PSUM -> SBUF bf16 copies (x_deq^T tiles).
  5. PE: bf16 GEMM, 8 k-block matmuls accumulating into PSUM per 512-col
     half; ACT copies PSUM -> SBUF fp32; batched DMA out.
"""

import numpy as np
import ml_dtypes
from contextlib import ExitStack

import concourse.bass as bass
import concourse.bacc as bacc
import concourse.mybir as mybir
import concourse.tile as tile
from concourse.bass_utils import run_bass_kernel_spmd
from concourse.bass_interp import get_hw_module

M, K, N = 65536, 1024, 1024
NCORES = 8
MS = M // NCORES          # 8192 rows per core
B = 128                   # quant block size
KB = K // B               # 8 k-blocks
NB = N // B
GROUP = 4                 # m-tiles per DMA batch (2 MB transfers)
FP8_HW_MAX = 224.0        # trn2 fp8e4 is IEEE e4m3 (max 240); 224 = 448/2

F32 = mybir.dt.float32
BF16 = mybir.dt.bfloat16
FP8 = mybir.dt.float8e4


def build_bass(ms: int = MS, group: int = GROUP):
    """Build + compile the per-core Bass program for an ms-row shard."""
    mt = ms // B                      # m-tiles
    ngroups = mt // group
    assert mt % group == 0

    nc = bacc.Bacc(
        "TRN2", target_bir_lowering=False, debug=False, num_devices=NCORES
    )
    x_d = nc.dram_tensor("x", [ms, K], F32, kind="ExternalInput")
    wt_d = nc.dram_tensor("wt", [K, N], BF16, kind="ExternalInput")
    id_d = nc.dram_tensor("ident", [B, B], BF16, kind="ExternalInput")
    y_d = nc.dram_tensor("y", [ms, N], F32, kind="ExternalOutput")

    x_r = x_d.ap().rearrange("(t p) k -> t p k", p=B)    # [mt, 128, 1024]
    y_r = y_d.ap().rearrange("(t p) n -> t p n", p=B)
    wt_r = wt_d.ap().rearrange("(kb p) n -> p kb n", p=B)

    with tile.TileContext(nc) as tc, ExitStack() as ctx:
        consts = ctx.enter_context(tc.tile_pool(name="consts", bufs=1))
        xin = ctx.enter_context(tc.tile_pool(name="xin", bufs=2))
        yout = ctx.enter_context(tc.tile_pool(name="yout", bufs=2))
        work = ctx.enter_context(tc.tile_pool(name="work", bufs=4))
        # all 8 PSUM banks: 1 m-tile of transposes in flight, 3 of y accum
        psum_t = ctx.enter_context(tc.tile_pool(name="psum_t", bufs=2, space="PSUM"))
        psum_y = ctx.enter_context(tc.tile_pool(name="psum_y", bufs=6, space="PSUM"))

        wt_s = consts.tile([B, KB, N], BF16)
        ident = consts.tile([B, B], BF16)
        nc.sync.dma_start(ident[:], id_d.ap())

        for g in range(ngroups):
            xt = xin.tile([B, group, K], F32, tag="xt")
            # per-m-tile loads: finer-grained prefetch, shorter pipeline head.
            # In group 0, only the first two tiles load up front; the rest
            # (and the 2 MB weight) are deferred behind the first reduce so
            # the m-tile-0 pipeline isn't starved by DMA round-robin.
            for j in range(2 if g == 0 else group):
                nc.sync.dma_start(xt[:, j], x_r[g * group + j])
            yt = yout.tile([B, group, N], F32, tag="yt")

            for j in range(group):
                xmk = xt[:, j]                           # [128, 1024] f32

                amax = work.tile([B, KB], F32, tag="amax")
                red_inst = nc.vector.tensor_reduce(
                    amax[:],
                    xmk.rearrange("p (kb b) -> p kb b", b=B),
                    axis=mybir.AxisListType.X,
                    op=mybir.AluOpType.max,
                    apply_absolute_value=True,
                )
                if g == 0 and j == 0:
                    for jj in range(2, group):
                        dep = nc.sync.dma_start(xt[:, jj], x_r[jj])
                        tile.add_dep_helper(
                            dep.ins, red_inst.ins, reason="defer load behind mtile0"
                        )
                    dep = nc.sync.dma_start(wt_s[:], wt_r)
                    tile.add_dep_helper(
                        dep.ins, red_inst.ins, reason="defer w load behind mtile0"
                    )
                # xs = max(amax, 448e-12) / 224  (one fused clamp+scale op)
                xs = work.tile([B, KB], F32, tag="xs")
                nc.vector.tensor_scalar(
                    xs[:], amax[:], 448e-12, 1.0 / FP8_HW_MAX,
                    op0=mybir.AluOpType.max, op1=mybir.AluOpType.mult,
                )
                rxs = work.tile([B, KB], F32, tag="rxs")
                nc.vector.reciprocal(rxs[:], xs[:])

                # quantize: xq = fp8e4(x * 224/amax) -- one broadcast multiply
                xq = work.tile([B, K], FP8, tag="xq")
                nc.vector.tensor_tensor(
                    xq[:].rearrange("p (kb b) -> p kb b", b=B),
                    xmk.rearrange("p (kb b) -> p kb b", b=B),
                    rxs[:, :, None].to_broadcast((B, KB, B)),
                    mybir.AluOpType.mult,
                )
                # diag(xs_kb) blocks = I * xs, built on the idle GpSimd engine
                diag8 = work.tile([B, KB, B], BF16, tag="diag8")
                nc.gpsimd.tensor_tensor(
                    diag8[:],
                    ident[:, None, :].to_broadcast((B, KB, B)),
                    xs[:, :, None].to_broadcast((B, KB, B)),
                    mybir.AluOpType.mult,
                )

                # transpose + dequant: psum[k, m] = sum_m' xq[m',k] diag[m',m]
                pt0 = psum_t.tile([B, 512], F32, tag="pt")
                pt1 = psum_t.tile([B, 512], F32, tag="pt")
                for kb in range(KB):
                    pt = pt0 if kb < 4 else pt1
                    nc.tensor.matmul(
                        pt[:, (kb % 4) * B:(kb % 4 + 1) * B],
                        xq[:, kb * B:(kb + 1) * B],
                        diag8[:, kb],
                        start=True,
                        stop=True,
                    )
                xT = work.tile([B, K], BF16, tag="xT")
                nc.scalar.copy(xT[:, 0:512], pt0[:])
                nc.scalar.copy(xT[:, 512:1024], pt1[:])

                # main GEMM: y[m, :] = sum_kb xT_kb^T @ wT[kb]
                py0 = psum_y.tile([B, 512], F32, tag="py")
                py1 = psum_y.tile([B, 512], F32, tag="py")
                for kb in range(KB):
                    lhsT = xT[:, kb * B:(kb + 1) * B]
                    nc.tensor.matmul(
                        py0[:], lhsT, wt_s[:, kb, 0:512],
                        start=(kb == 0), stop=(kb == KB - 1),
                    )
                    nc.tensor.matmul(
                        py1[:], lhsT, wt_s[:, kb, 512:1024],
                        start=(kb == 0), stop=(kb == KB - 1),
                    )
                nc.scalar.copy(yt[:, j, 0:512], py0[:])
                nc.scalar.copy(yt[:, j, 512:1024], py1[:])
                nc.sync.dma_start(y_r[g * group + j], yt[:, j])

    nc.compile()
    nc.m = get_hw_module(nc.m)
    return nc


def host_prep(weight, w_scale):
    weight = np.asarray(weight)
    if weight.dtype != ml_dtypes.float8_e4m3fn:
        weight = weight.view(ml_dtypes.float8_e4m3fn)
    w_scale = np.asarray(w_scale, dtype=np.float32)
    nb, kb = w_scale.shape
    w_deq = (
        weight.astype(np.float32).reshape(nb, B, kb, B)
        * w_scale[:, None, :, None]
    ).reshape(nb * B, kb * B)
    wt = np.ascontiguousarray(w_deq.T).astype(ml_dtypes.bfloat16)  # [K, N]
    ident = np.eye(B, dtype=ml_dtypes.bfloat16)
    return wt, ident


_NC_CACHE = {}


def _get_nc(ms):
    if ms not in _NC_CACHE:
        _NC_CACHE[ms] = build_bass(ms)
    return _NC_CACHE[ms]


def kernel(x, weight, w_scale, _trace=False):
    x = np.ascontiguousarray(np.asarray(x, dtype=np.float32))
    assert x.shape == (M, K), x.shape
    wt, ident = host_prep(weight, w_scale)
    nc = _get_nc(MS)
    in_maps = [
        {"x": x[c * MS:(c + 1) * MS], "wt": wt, "ident": ident}
        for c in range(NCORES)
    ]
    res = run_bass_kernel_spmd(
        nc, in_maps, core_ids=list(range(NCORES)), trace=_trace
    )
    y = np.concatenate([r["y"] for r in res.results], axis=0)
    if _trace:
        return y, res
    return y


# revision 8
# speedup vs baseline: 1.2401x; 1.2401x over previous
"""BlockwiseQuantLinear Trainium2 kernel.

y = x_deq @ w_deq.T where
  x_deq = fp8-blockwise-quantize-dequantize(x)  (1x128 blocks along K)
  w_deq = fp8 weight * 128x128 blockwise scales

Strategy: data-parallel over M across the 8 NeuronCores (8192 rows each).
Weight is dequantized to bf16 on host (small: 1024x1024), transposed to
[K, N], and replicated. On device, per 128-row m-tile:
  1. DVE: segmented abs-max over 1x128 blocks -> per-(row,block) scales.
     TRN2's fp8e4 is IEEE e4m3 (max 240), not e4m3fn (max 448), so we
     quantize with 224/amax: identical RTNE rounding up to a power of two.
  2. DVE: x * (224/amax) -> xq in fp8e4, and diag matrices diag(amax/224).
  3. PE: xq_block^T @ diag -> PSUM, which both transposes the block (K on
     partitions) and applies the dequant scale in one matmul.
  4. ACT: PSUM -> SBUF bf16 copies (x_deq^T tiles).
  5. PE: bf16 GEMM, 8 k-block matmuls accumulating into PSUM per 512-col
     half; ACT copies PSUM -> SBUF fp32; batched DMA out.
"""

import numpy as np
import ml_dtypes
from contextlib import ExitStack

import concourse.bass as bass
import concourse.bacc as bacc
import concourse.mybir as mybir
import concourse.tile as tile
from concourse.bass_utils import run_bass_kernel_spmd
from concourse.bass_interp import get_hw_module

M, K, N = 65536, 1024, 1024
NCORES = 8
MS = M // NCORES          # 8192 rows per core
B = 128                   # quant block size
KB = K // B               # 8 k-blocks
NB = N // B
GROUP = 4                 # m-tiles per DMA batch (2 MB transfers)
FP8_HW_MAX = 224.0        # trn2 fp8e4 is IEEE e4m3 (max 240); 224 = 448/2

F32 = mybir.dt.float32
BF16 = mybir.dt.bfloat16
FP8 = mybir.dt.float8e4


def build_bass(ms: int = MS, group: int = GROUP):
    """Build + compile the per-core Bass program for an ms-row shard."""
    mt = ms // B                      # m-tiles
    ngroups = mt // group
    assert mt % group == 0

    nc = bacc.Bacc(
        "TRN2", target_bir_lowering=False, debug=False, num_devices=NCORES
    )
    x_d = nc.dram_tensor("x", [ms, K], F32, kind="ExternalInput")
    wt_d = nc.dram_tensor("wt", [K, N], BF16, kind="ExternalInput")
    id_d = nc.dram_tensor("ident", [B, B], BF16, kind="ExternalInput")
    y_d = nc.dram_tensor("y", [ms, N], F32, kind="ExternalOutput")

    x_r = x_d.ap().rearrange("(t p) k -> t p k", p=B)    # [mt, 128, 1024]
    y_r = y_d.ap().rearrange("(t p) n -> t p n", p=B)
    wt_r = wt_d.ap().rearrange("(kb p) n -> p kb n", p=B)

    with tile.TileContext(nc) as tc, ExitStack() as ctx:
        consts = ctx.enter_context(tc.tile_pool(name="consts", bufs=1))
        xin = ctx.enter_context(tc.tile_pool(name="xin", bufs=2))
        yout = ctx.enter_context(tc.tile_pool(name="yout", bufs=2))
        work = ctx.enter_context(tc.tile_pool(name="work", bufs=4))
        psum_t = ctx.enter_context(tc.tile_pool(name="psum_t", bufs=4, space="PSUM"))
        psum_y = ctx.enter_context(tc.tile_pool(name="psum_y", bufs=4, space="PSUM"))

        wt_s = consts.tile([B, KB, N], BF16)
        ident = consts.tile([B, B], BF16)
        nc.sync.dma_start(ident[:], id_d.ap())

        xts = {}
        yts = {}

        def get_xt(g):
            # per-m-tile loads: finer-grained prefetch, shorter pipeline head.
            # In group 0, only the first two tiles load up front; the rest
            # (and the 2 MB weight) are deferred behind the first reduce so
            # the m-tile-0 pipeline isn't starved by DMA round-robin.
            if g not in xts:
                xt = xin.tile([B, group, K], F32, tag="xt", name="xt")
                for j in range(2 if g == 0 else group):
                    nc.sync.dma_start(xt[:, j], x_r[g * group + j])
                xts[g] = xt
            return xts[g]

        def stage_a(t):
            """quant + transpose-dequant: produces xT (bf16 [k, m]) for tile t."""
            g, j = divmod(t, group)
            xmk = get_xt(g)[:, j]                        # [128, 1024] f32

            amax = work.tile([B, KB], F32, tag="amax", name="amax")
            red_inst = nc.vector.tensor_reduce(
                amax[:],
                xmk.rearrange("p (kb b) -> p kb b", b=B),
                axis=mybir.AxisListType.X,
                op=mybir.AluOpType.max,
                apply_absolute_value=True,
            )
            if t == 0:
                for jj in range(2, group):
                    dep = nc.sync.dma_start(get_xt(0)[:, jj], x_r[jj])
                    tile.add_dep_helper(
                        dep.ins, red_inst.ins, reason="defer load behind mtile0"
                    )
                dep = nc.sync.dma_start(wt_s[:], wt_r)
                tile.add_dep_helper(
                    dep.ins, red_inst.ins, reason="defer w load behind mtile0"
                )
            # xs = max(amax, 448e-12) / 224  (one fused clamp+scale op)
            xs = work.tile([B, KB], F32, tag="xs", name="xs")
            nc.vector.tensor_scalar(
                xs[:], amax[:], 448e-12, 1.0 / FP8_HW_MAX,
                op0=mybir.AluOpType.max, op1=mybir.AluOpType.mult,
            )
            rxs = work.tile([B, KB], F32, tag="rxs", name="rxs")
            nc.vector.reciprocal(rxs[:], xs[:])

            # quantize: xq = fp8e4(x * 224/amax) -- one broadcast multiply
            xq = work.tile([B, K], FP8, tag="xq", name="xq")
            nc.vector.tensor_tensor(
                xq[:].rearrange("p (kb b) -> p kb b", b=B),
                xmk.rearrange("p (kb b) -> p kb b", b=B),
                rxs[:, :, None].to_broadcast((B, KB, B)),
                mybir.AluOpType.mult,
            )
            # diag(xs_kb) blocks = I * xs, built on the idle GpSimd engine
            diag8 = work.tile([B, KB, B], BF16, tag="diag8", name="diag8")
            nc.gpsimd.tensor_tensor(
                diag8[:],
                ident[:, None, :].to_broadcast((B, KB, B)),
                xs[:, :, None].to_broadcast((B, KB, B)),
                mybir.AluOpType.mult,
            )

            # transpose + dequant: psum[k, m] = sum_m' xq[m',k] diag[m',m]
            pt0 = psum_t.tile([B, 512], F32, tag="pt", name="pt0")
            pt1 = psum_t.tile([B, 512], F32, tag="pt", name="pt1")
            for kb in range(KB):
                pt = pt0 if kb < 4 else pt1
                nc.tensor.matmul(
                    pt[:, (kb % 4) * B:(kb % 4 + 1) * B],
                    xq[:, kb * B:(kb + 1) * B],
                    diag8[:, kb],
                    start=True,
                    stop=True,
                )
            xT = work.tile([B, K], BF16, tag="xT", name="xT")
            nc.scalar.copy(xT[:, 0:512], pt0[:])
            nc.scalar.copy(xT[:, 512:1024], pt1[:])
            return xT

        def stage_b(t, xT):
            """main GEMM + output copy/DMA for tile t."""
            g, j = divmod(t, group)
            if g not in yts:
                yts[g] = yout.tile([B, group, N], F32, tag="yt", name="yt")
            yt = yts[g]
            py0 = psum_y.tile([B, 512], F32, tag="py", name="py0")
            py1 = psum_y.tile([B, 512], F32, tag="py", name="py1")
            for kb in range(KB):
                lhsT = xT[:, kb * B:(kb + 1) * B]
                nc.tensor.matmul(
                    py0[:], lhsT, wt_s[:, kb, 0:512],
                    start=(kb == 0), stop=(kb == KB - 1),
                )
                nc.tensor.matmul(
                    py1[:], lhsT, wt_s[:, kb, 512:1024],
                    start=(kb == 0), stop=(kb == KB - 1),
                )
            nc.scalar.copy(yt[:, j, 0:512], py0[:])
            nc.scalar.copy(yt[:, j, 512:1024], py1[:])
            nc.sync.dma_start(y_r[t], yt[:, j])

        # software pipeline: transposes of tile t+1 are emitted before the
        # GEMM of tile t, so the PE always has independent fill work when a
        # GEMM briefly waits on PSUM recycling.
        prev = None
        for t in range(mt):
            xT = stage_a(t)
            if prev is not None:
                stage_b(prev[0], prev[1])
            prev = (t, xT)
        stage_b(prev[0], prev[1])

    nc.compile()
    nc.m = get_hw_module(nc.m)
    return nc


def host_prep(weight, w_scale):
    weight = np.asarray(weight)
    if weight.dtype != ml_dtypes.float8_e4m3fn:
        weight = weight.view(ml_dtypes.float8_e4m3fn)
    w_scale = np.asarray(w_scale, dtype=np.float32)
    nb, kb = w_scale.shape
    w_deq = (
        weight.astype(np.float32).reshape(nb, B, kb, B)
        * w_scale[:, None, :, None]
    ).reshape(nb * B, kb * B)
    wt = np.ascontiguousarray(w_deq.T).astype(ml_dtypes.bfloat16)  # [K, N]
    ident = np.eye(B, dtype=ml_dtypes.bfloat16)
    return wt, ident


_NC_CACHE = {}


def _get_nc(ms):
    if ms not in _NC_CACHE:
        _NC_CACHE[ms] = build_bass(ms)
    return _NC_CACHE[ms]


def kernel(x, weight, w_scale, _trace=False):
    x = np.ascontiguousarray(np.asarray(x, dtype=np.float32))
    assert x.shape == (M, K), x.shape
    wt, ident = host_prep(weight, w_scale)
    nc = _get_nc(MS)
    in_maps = [
        {"x": x[c * MS:(c + 1) * MS], "wt": wt, "ident": ident}
        for c in range(NCORES)
    ]
    res = run_bass_kernel_spmd(
        nc, in_maps, core_ids=list(range(NCORES)), trace=_trace
    )
    y = np.concatenate([r["y"] for r in res.results], axis=0)
    if _trace:
        return y, res
    return y


# revision 11
# speedup vs baseline: 1.2428x; 1.0022x over previous
"""BlockwiseQuantLinear Trainium2 kernel.

y = x_deq @ w_deq.T where
  x_deq = fp8-blockwise-quantize-dequantize(x)  (1x128 blocks along K)
  w_deq = fp8 weight * 128x128 blockwise scales

Strategy: data-parallel over M across the 8 NeuronCores (8192 rows each).
Weight is dequantized to bf16 on host (small: 1024x1024), transposed to
[K, N], and replicated. On device, per 128-row m-tile:
  1. DVE: segmented abs-max over 1x128 blocks -> per-(row,block) scales.
     TRN2's fp8e4 is IEEE e4m3 (max 240), not e4m3fn (max 448), so we
     quantize with 224/amax: identical RTNE rounding up to a power of two.
  2. DVE: x * (224/amax) -> xq in fp8e4, and diag matrices diag(amax/224).
  3. PE: xq_block^T @ diag -> PSUM, which both transposes the block (K on
     partitions) and applies the dequant scale in one matmul.
  4. ACT: PSUM -> SBUF bf16 copies (x_deq^T tiles).
  5. PE: bf16 GEMM, 8 k-block matmuls accumulating into PSUM per 512-col
     half; ACT copies PSUM -> SBUF fp32; batched DMA out.
"""

import numpy as np
import ml_dtypes
from contextlib import ExitStack

import concourse.bass as bass
import concourse.bacc as bacc
import concourse.mybir as mybir
import concourse.tile as tile
from concourse.bass_utils import run_bass_kernel_spmd
from concourse.bass_interp import get_hw_module

M, K, N = 65536, 1024, 1024
NCORES = 8
MS = M // NCORES          # 8192 rows per core
B = 128                   # quant block size
KB = K // B               # 8 k-blocks
NB = N // B
GROUP = 4                 # m-tiles per DMA batch (2 MB transfers)
FP8_HW_MAX = 224.0        # trn2 fp8e4 is IEEE e4m3 (max 240); 224 = 448/2

F32 = mybir.dt.float32
BF16 = mybir.dt.bfloat16
FP8 = mybir.dt.float8e4


def build_bass(ms: int = MS, group: int = GROUP):
    """Build + compile the per-core Bass program for an ms-row shard."""
    mt = ms // B                      # m-tiles
    ngroups = mt // group
    assert mt % group == 0

    nc = bacc.Bacc(
        "TRN2", target_bir_lowering=False, debug=False, num_devices=NCORES
    )
    x_d = nc.dram_tensor("x", [ms, K], F32, kind="ExternalInput")
    wt_d = nc.dram_tensor("wt", [K, N], BF16, kind="ExternalInput")
    id_d = nc.dram_tensor("ident", [B, B], BF16, kind="ExternalInput")
    y_d = nc.dram_tensor("y", [ms, N], F32, kind="ExternalOutput")

    x_r = x_d.ap().rearrange("(t p) k -> t p k", p=B)    # [mt, 128, 1024]
    y_r = y_d.ap().rearrange("(t p) n -> t p n", p=B)
    wt_r = wt_d.ap().rearrange("(kb p) n -> p kb n", p=B)

    with tile.TileContext(nc) as tc, ExitStack() as ctx:
        consts = ctx.enter_context(tc.tile_pool(name="consts", bufs=1))
        xin = ctx.enter_context(tc.tile_pool(name="xin", bufs=2))
        yout = ctx.enter_context(tc.tile_pool(name="yout", bufs=2))
        work = ctx.enter_context(tc.tile_pool(name="work", bufs=6))
        psum_t = ctx.enter_context(tc.tile_pool(name="psum_t", bufs=4, space="PSUM"))
        psum_y = ctx.enter_context(tc.tile_pool(name="psum_y", bufs=4, space="PSUM"))

        wt_s = consts.tile([B, KB, N], BF16)
        ident = consts.tile([B, B], BF16)
        nc.sync.dma_start(ident[:], id_d.ap())
        nc.sync.dma_start(wt_s[:, 0], wt_r[:, 0])

        xts = {}
        yts = {}

        def get_xt(g):
            # per-m-tile loads: finer-grained prefetch, shorter pipeline head.
            # In group 0, only the first two tiles load up front; the rest
            # (and the 2 MB weight) are deferred behind the first reduce so
            # the m-tile-0 pipeline isn't starved by DMA round-robin.
            if g not in xts:
                xt = xin.tile([B, group, K], F32, tag="xt", name="xt")
                for j in range(2 if g == 0 else group):
                    nc.sync.dma_start(xt[:, j], x_r[g * group + j])
                xts[g] = xt
            return xts[g]

        def stage_a(t):
            """quant + transpose-dequant: produces xT (bf16 [k, m]) for tile t."""
            g, j = divmod(t, group)
            xmk = get_xt(g)[:, j]                        # [128, 1024] f32

            amax = work.tile([B, KB], F32, tag="amax", name="amax")
            red_inst = nc.vector.tensor_reduce(
                amax[:],
                xmk.rearrange("p (kb b) -> p kb b", b=B),
                axis=mybir.AxisListType.X,
                op=mybir.AluOpType.max,
                apply_absolute_value=True,
            )
            if t == 0:
                for jj in range(2, group):
                    dep = nc.sync.dma_start(get_xt(0)[:, jj], x_r[jj])
                    tile.add_dep_helper(
                        dep.ins, red_inst.ins, reason="defer load behind mtile0"
                    )
                # weight k-block 0 loads up front (first GEMM matmul needs
                # only it); the remaining 7/8 of the weight follows behind
                # the first reduce so it doesn't starve the m-tile-0 chain.
                dep = nc.sync.dma_start(wt_s[:, 1:], wt_r[:, 1:])
                tile.add_dep_helper(
                    dep.ins, red_inst.ins, reason="defer w load behind mtile0"
                )
            # xs = max(amax, 448e-12) / 224  (one fused clamp+scale op)
            xs = work.tile([B, KB], F32, tag="xs", name="xs")
            nc.vector.tensor_scalar(
                xs[:], amax[:], 448e-12, 1.0 / FP8_HW_MAX,
                op0=mybir.AluOpType.max, op1=mybir.AluOpType.mult,
            )
            rxs = work.tile([B, KB], F32, tag="rxs", name="rxs")
            nc.vector.reciprocal(rxs[:], xs[:])

            # quantize: xq = fp8e4(x * 224/amax) -- one broadcast multiply
            xq = work.tile([B, K], FP8, tag="xq", name="xq")
            nc.vector.tensor_tensor(
                xq[:].rearrange("p (kb b) -> p kb b", b=B),
                xmk.rearrange("p (kb b) -> p kb b", b=B),
                rxs[:, :, None].to_broadcast((B, KB, B)),
                mybir.AluOpType.mult,
            )
            # diag(xs_kb) blocks = I * xs, built on the idle GpSimd engine
            diag8 = work.tile([B, KB, B], BF16, tag="diag8", name="diag8")
            nc.gpsimd.tensor_tensor(
                diag8[:],
                ident[:, None, :].to_broadcast((B, KB, B)),
                xs[:, :, None].to_broadcast((B, KB, B)),
                mybir.AluOpType.mult,
            )

            # transpose + dequant: psum[k, m] = sum_m' xq[m',k] diag[m',m]
            pt0 = psum_t.tile([B, 512], F32, tag="pt", name="pt0")
            pt1 = psum_t.tile([B, 512], F32, tag="pt", name="pt1")
            for kb in range(KB):
                pt = pt0 if kb < 4 else pt1
                nc.tensor.matmul(
                    pt[:, (kb % 4) * B:(kb % 4 + 1) * B],
                    xq[:, kb * B:(kb + 1) * B],
                    diag8[:, kb],
                    start=True,
                    stop=True,
                )
            xT = work.tile([B, K], BF16, tag="xT", name="xT")
            nc.scalar.copy(xT[:, 0:512], pt0[:])
            nc.scalar.copy(xT[:, 512:1024], pt1[:])
            return xT

        def stage_b(t, xT):
            """main GEMM + output copy/DMA for tile t."""
            g, j = divmod(t, group)
            if g not in yts:
                yts[g] = yout.tile([B, group, N], F32, tag="yt", name="yt")
            yt = yts[g]
            py0 = psum_y.tile([B, 512], F32, tag="py", name="py0")
            py1 = psum_y.tile([B, 512], F32, tag="py", name="py1")
            for kb in range(KB):
                lhsT = xT[:, kb * B:(kb + 1) * B]
                nc.tensor.matmul(
                    py0[:], lhsT, wt_s[:, kb, 0:512],
                    start=(kb == 0), stop=(kb == KB - 1),
                )
                nc.tensor.matmul(
                    py1[:], lhsT, wt_s[:, kb, 512:1024],
                    start=(kb == 0), stop=(kb == KB - 1),
                )
            nc.scalar.copy(yt[:, j, 0:512], py0[:])
            nc.scalar.copy(yt[:, j, 512:1024], py1[:])
            nc.sync.dma_start(y_r[t], yt[:, j])

        # software pipeline: transposes of tile t+1 are emitted before the
        # GEMM of tile t, so the PE always has independent fill work when a
        # GEMM briefly waits on PSUM recycling.
        prev = None
        for t in range(mt):
            xT = stage_a(t)
            if prev is not None:
                stage_b(prev[0], prev[1])
            prev = (t, xT)
        stage_b(prev[0], prev[1])

    nc.compile()
    nc.m = get_hw_module(nc.m)
    return nc


def host_prep(weight, w_scale):
    weight = np.asarray(weight)
    if weight.dtype != ml_dtypes.float8_e4m3fn:
        weight = weight.view(ml_dtypes.float8_e4m3fn)
    w_scale = np.asarray(w_scale, dtype=np.float32)
    nb, kb = w_scale.shape
    w_deq = (
        weight.astype(np.float32).reshape(nb, B, kb, B)
        * w_scale[:, None, :, None]
    ).reshape(nb * B, kb * B)
    wt = np.ascontiguousarray(w_deq.T).astype(ml_dtypes.bfloat16)  # [K, N]
    ident = np.eye(B, dtype=ml_dtypes.bfloat16)
    return wt, ident


_NC_CACHE = {}


def _get_nc(ms):
    if ms not in _NC_CACHE:
        _NC_CACHE[ms] = build_bass(ms)
    return _NC_CACHE[ms]


def kernel(x, weight, w_scale, _trace=False):
    x = np.ascontiguousarray(np.asarray(x, dtype=np.float32))
    assert x.shape == (M, K), x.shape
    wt, ident = host_prep(weight, w_scale)
    nc = _get_nc(MS)
    in_maps = [
        {"x": x[c * MS:(c + 1) * MS], "wt": wt, "ident": ident}
        for c in range(NCORES)
    ]
    res = run_bass_kernel_spmd(
        nc, in_maps, core_ids=list(range(NCORES)), trace=_trace
    )
    y = np.concatenate([r["y"] for r in res.results], axis=0)
    if _trace:
        return y, res
    return y


# revision 13
# speedup vs baseline: 1.2759x; 1.0266x over previous
"""BlockwiseQuantLinear Trainium2 kernel.

y = x_deq @ w_deq.T where
  x_deq = fp8-blockwise-quantize-dequantize(x)  (1x128 blocks along K)
  w_deq = fp8 weight * 128x128 blockwise scales

Strategy: data-parallel over M across the 8 NeuronCores (8192 rows each).
Weight is dequantized to bf16 on host (small: 1024x1024), transposed to
[K, N], and replicated. On device, per 128-row m-tile:
  1. DVE: segmented abs-max over 1x128 blocks -> per-(row,block) scales.
     TRN2's fp8e4 is IEEE e4m3 (max 240), not e4m3fn (max 448), so we
     quantize with 224/amax: identical RTNE rounding up to a power of two.
  2. DVE: x * (224/amax) -> xq in fp8e4, and diag matrices diag(amax/224).
  3. PE: xq_block^T @ diag -> PSUM, which both transposes the block (K on
     partitions) and applies the dequant scale in one matmul.
  4. ACT: PSUM -> SBUF bf16 copies (x_deq^T tiles).
  5. PE: bf16 GEMM, 8 k-block matmuls accumulating into PSUM per 512-col
     half; ACT copies PSUM -> SBUF fp32; batched DMA out.
"""

import numpy as np
import ml_dtypes
from contextlib import ExitStack

import concourse.bass as bass
import concourse.bacc as bacc
import concourse.mybir as mybir
import concourse.tile as tile
from concourse.bass_utils import run_bass_kernel_spmd
from concourse.bass_interp import get_hw_module

M, K, N = 65536, 1024, 1024
NCORES = 8
MS = M // NCORES          # 8192 rows per core
B = 128                   # quant block size
KB = K // B               # 8 k-blocks
NB = N // B
GROUP = 4                 # m-tiles per DMA batch (2 MB transfers)
FP8_HW_MAX = 224.0        # trn2 fp8e4 is IEEE e4m3 (max 240); 224 = 448/2

F32 = mybir.dt.float32
BF16 = mybir.dt.bfloat16
FP8 = mybir.dt.float8e4


def build_bass(ms: int = MS, group: int = GROUP):
    """Build + compile the per-core Bass program for an ms-row shard."""
    mt = ms // B                      # m-tiles
    ngroups = mt // group
    assert mt % group == 0

    nc = bacc.Bacc(
        "TRN2", target_bir_lowering=False, debug=False, num_devices=NCORES
    )
    x_d = nc.dram_tensor("x", [ms, K], F32, kind="ExternalInput")
    wt_d = nc.dram_tensor("wt", [K, N], BF16, kind="ExternalInput")
    id_d = nc.dram_tensor("ident", [B, B], BF16, kind="ExternalInput")
    y_d = nc.dram_tensor("y", [ms, N], F32, kind="ExternalOutput")

    x_r = x_d.ap().rearrange("(t p) k -> t p k", p=B)    # [mt, 128, 1024]
    y_r = y_d.ap().rearrange("(t p) n -> t p n", p=B)
    wt_r = wt_d.ap().rearrange("(kb p) n -> p kb n", p=B)

    with tile.TileContext(nc) as tc, ExitStack() as ctx:
        consts = ctx.enter_context(tc.tile_pool(name="consts", bufs=1))
        xin = ctx.enter_context(tc.tile_pool(name="xin", bufs=2))
        yout = ctx.enter_context(tc.tile_pool(name="yout", bufs=2))
        work = ctx.enter_context(tc.tile_pool(name="work", bufs=6))
        psum_t = ctx.enter_context(tc.tile_pool(name="psum_t", bufs=4, space="PSUM"))
        psum_y = ctx.enter_context(tc.tile_pool(name="psum_y", bufs=4, space="PSUM"))

        wt_s = consts.tile([B, KB, N], BF16)
        ident = consts.tile([B, B], BF16)
        nc.sync.dma_start(ident[:], id_d.ap())
        nc.sync.dma_start(wt_s[:, 0], wt_r[:, 0])

        xts = {}
        yts = {}

        def get_xt(g):
            # per-m-tile loads: finer-grained prefetch, shorter pipeline head.
            # In group 0, only the first two tiles load up front; the rest
            # (and the 2 MB weight) are deferred behind the first reduce so
            # the m-tile-0 pipeline isn't starved by DMA round-robin.
            if g not in xts:
                xt = xin.tile([B, group, K], F32, tag="xt", name="xt")
                for j in range(2 if g == 0 else group):
                    nc.sync.dma_start(xt[:, j], x_r[g * group + j])
                xts[g] = xt
            return xts[g]

        def stage_a(t):
            """quant + transpose-dequant: produces xT (bf16 [k, m]) for tile t."""
            g, j = divmod(t, group)
            xmk = get_xt(g)[:, j]                        # [128, 1024] f32

            amax = work.tile([B, KB], F32, tag="amax", name="amax")
            red_inst = nc.vector.tensor_reduce(
                amax[:],
                xmk.rearrange("p (kb b) -> p kb b", b=B),
                axis=mybir.AxisListType.X,
                op=mybir.AluOpType.max,
                apply_absolute_value=True,
            )
            if t == 0:
                for jj in range(2, group):
                    dep = nc.sync.dma_start(get_xt(0)[:, jj], x_r[jj])
                    tile.add_dep_helper(
                        dep.ins, red_inst.ins, reason="defer load behind mtile0"
                    )
                # weight k-block 0 loads up front (first GEMM matmul needs
                # only it); the remaining 7/8 of the weight follows behind
                # the first reduce so it doesn't starve the m-tile-0 chain.
                dep = nc.sync.dma_start(wt_s[:, 1:], wt_r[:, 1:])
                tile.add_dep_helper(
                    dep.ins, red_inst.ins, reason="defer w load behind mtile0"
                )
            # xs = max(amax, 448e-12) / 224  (one fused clamp+scale op)
            xs = work.tile([B, KB], F32, tag="xs", name="xs")
            nc.vector.tensor_scalar(
                xs[:], amax[:], 448e-12, 1.0 / FP8_HW_MAX,
                op0=mybir.AluOpType.max, op1=mybir.AluOpType.mult,
            )
            rxs = work.tile([B, KB], F32, tag="rxs", name="rxs")
            nc.vector.reciprocal(rxs[:], xs[:])

            # quantize: xq = fp8e4(x * 224/amax) -- one broadcast multiply
            xq = work.tile([B, K], FP8, tag="xq", name="xq")
            nc.vector.tensor_tensor(
                xq[:].rearrange("p (kb b) -> p kb b", b=B),
                xmk.rearrange("p (kb b) -> p kb b", b=B),
                rxs[:, :, None].to_broadcast((B, KB, B)),
                mybir.AluOpType.mult,
            )
            # diag(xs_kb) blocks = I * xs. Cheap bf16 4x-mode ops on DVE;
            # keeping GpSimd idle avoids the DVE<->GpSimd SBUF-port lock
            # that would halve DVE throughput while GpSimd streams.
            diag8 = work.tile([B, KB, B], BF16, tag="diag8", name="diag8")
            for kb in range(KB):
                nc.vector.tensor_scalar_mul(
                    diag8[:, kb], ident[:], xs[:, kb:kb + 1]
                )

            # transpose + dequant: psum[k, m] = sum_m' xq[m',k] diag[m',m]
            pt0 = psum_t.tile([B, 512], F32, tag="pt", name="pt0")
            pt1 = psum_t.tile([B, 512], F32, tag="pt", name="pt1")
            for kb in range(KB):
                pt = pt0 if kb < 4 else pt1
                nc.tensor.matmul(
                    pt[:, (kb % 4) * B:(kb % 4 + 1) * B],
                    xq[:, kb * B:(kb + 1) * B],
                    diag8[:, kb],
                    start=True,
                    stop=True,
                )
            xT = work.tile([B, K], BF16, tag="xT", name="xT")
            nc.scalar.copy(xT[:, 0:512], pt0[:])
            nc.scalar.copy(xT[:, 512:1024], pt1[:])
            return xT

        def stage_b(t, xT):
            """main GEMM + output copy/DMA for tile t."""
            g, j = divmod(t, group)
            if g not in yts:
                yts[g] = yout.tile([B, group, N], F32, tag="yt", name="yt")
            yt = yts[g]
            py0 = psum_y.tile([B, 512], F32, tag="py", name="py0")
            py1 = psum_y.tile([B, 512], F32, tag="py", name="py1")
            for kb in range(KB):
                lhsT = xT[:, kb * B:(kb + 1) * B]
                nc.tensor.matmul(
                    py0[:], lhsT, wt_s[:, kb, 0:512],
                    start=(kb == 0), stop=(kb == KB - 1),
                )
                nc.tensor.matmul(
                    py1[:], lhsT, wt_s[:, kb, 512:1024],
                    start=(kb == 0), stop=(kb == KB - 1),
                )
            nc.scalar.copy(yt[:, j, 0:512], py0[:])
            nc.scalar.copy(yt[:, j, 512:1024], py1[:])
            nc.sync.dma_start(y_r[t], yt[:, j])

        # software pipeline (depth 2): transposes run up to two m-tiles
        # ahead of the GEMM, so the PE always has independent fill work
        # when a GEMM briefly waits on PSUM recycling or ACT copies.
        from collections import deque
        pending = deque()
        for t in range(mt):
            pending.append((t, stage_a(t)))
            if len(pending) > 2:
                stage_b(*pending.popleft())
        while pending:
            stage_b(*pending.popleft())

    nc.compile()
    nc.m = get_hw_module(nc.m)
    return nc


def host_prep(weight, w_scale):
    weight = np.asarray(weight)
    if weight.dtype != ml_dtypes.float8_e4m3fn:
        weight = weight.view(ml_dtypes.float8_e4m3fn)
    w_scale = np.asarray(w_scale, dtype=np.float32)
    nb, kb = w_scale.shape
    w_deq = (
        weight.astype(np.float32).reshape(nb, B, kb, B)
        * w_scale[:, None, :, None]
    ).reshape(nb * B, kb * B)
    wt = np.ascontiguousarray(w_deq.T).astype(ml_dtypes.bfloat16)  # [K, N]
    ident = np.eye(B, dtype=ml_dtypes.bfloat16)
    return wt, ident


_NC_CACHE = {}


def _get_nc(ms):
    if ms not in _NC_CACHE:
        _NC_CACHE[ms] = build_bass(ms)
    return _NC_CACHE[ms]


def kernel(x, weight, w_scale, _trace=False):
    x = np.ascontiguousarray(np.asarray(x, dtype=np.float32))
    assert x.shape == (M, K), x.shape
    wt, ident = host_prep(weight, w_scale)
    nc = _get_nc(MS)
    in_maps = [
        {"x": x[c * MS:(c + 1) * MS], "wt": wt, "ident": ident}
        for c in range(NCORES)
    ]
    res = run_bass_kernel_spmd(
        nc, in_maps, core_ids=list(range(NCORES)), trace=_trace
    )
    y = np.concatenate([r["y"] for r in res.results], axis=0)
    if _trace:
        return y, res
    return y


# revision 15
# speedup vs baseline: 1.3328x; 1.0447x over previous
"""BlockwiseQuantLinear Trainium2 kernel.

y = x_deq @ w_deq.T where
  x_deq = fp8-blockwise-quantize-dequantize(x)  (1x128 blocks along K)
  w_deq = fp8 weight * 128x128 blockwise scales

Strategy: data-parallel over M across the 8 NeuronCores (8192 rows each).
Weight is dequantized to bf16 on host (small: 1024x1024), transposed to
[K, N], and replicated. On device, per 128-row m-tile:
  1. DVE: segmented abs-max over 1x128 blocks -> per-(row,block) scales.
     TRN2's fp8e4 is IEEE e4m3 (max 240), not e4m3fn (max 448), so we
     quantize with 224/amax: identical RTNE rounding up to a power of two.
  2. DVE: x * (224/amax) -> xq in fp8e4, and diag matrices diag(amax/224).
  3. PE: xq_block^T @ diag -> PSUM, which both transposes the block (K on
     partitions) and applies the dequant scale in one matmul.
  4. ACT: PSUM -> SBUF bf16 copies (x_deq^T tiles).
  5. PE: bf16 GEMM, 8 k-block matmuls accumulating into PSUM per 512-col
     half; ACT copies PSUM -> SBUF fp32; batched DMA out.
"""

import numpy as np
import ml_dtypes
from contextlib import ExitStack

import concourse.bass as bass
import concourse.bacc as bacc
import concourse.mybir as mybir
import concourse.tile as tile
from concourse.bass_utils import run_bass_kernel_spmd
from concourse.bass_interp import get_hw_module

M, K, N = 65536, 1024, 1024
NCORES = 8
MS = M // NCORES          # 8192 rows per core
B = 128                   # quant block size
KB = K // B               # 8 k-blocks
NB = N // B
GROUP = 4                 # m-tiles per DMA batch (2 MB transfers)
FP8_HW_MAX = 224.0        # trn2 fp8e4 is IEEE e4m3 (max 240); 224 = 448/2

F32 = mybir.dt.float32
BF16 = mybir.dt.bfloat16
FP8 = mybir.dt.float8e4


def build_bass(ms: int = MS, group: int = GROUP):
    """Build + compile the per-core Bass program for an ms-row shard."""
    mt = ms // B                      # m-tiles
    ngroups = mt // group
    assert mt % group == 0

    nc = bacc.Bacc(
        "TRN2", target_bir_lowering=False, debug=False, num_devices=NCORES
    )
    x_d = nc.dram_tensor("x", [ms, K], F32, kind="ExternalInput")
    wt_d = nc.dram_tensor("wt", [K, N], BF16, kind="ExternalInput")
    id_d = nc.dram_tensor("ident", [B, B], BF16, kind="ExternalInput")
    y_d = nc.dram_tensor("y", [ms, N], F32, kind="ExternalOutput")

    x_r = x_d.ap().rearrange("(t p) k -> t p k", p=B)    # [mt, 128, 1024]
    y_r = y_d.ap().rearrange("(t p) n -> t p n", p=B)
    wt_r = wt_d.ap().rearrange("(kb p) n -> p kb n", p=B)

    with tile.TileContext(nc) as tc, ExitStack() as ctx:
        consts = ctx.enter_context(tc.tile_pool(name="consts", bufs=1))
        xin = ctx.enter_context(tc.tile_pool(name="xin", bufs=3))
        yout = ctx.enter_context(tc.tile_pool(name="yout", bufs=3))
        work = ctx.enter_context(tc.tile_pool(name="work", bufs=6))
        psum_t = ctx.enter_context(tc.tile_pool(name="psum_t", bufs=4, space="PSUM"))
        psum_y = ctx.enter_context(tc.tile_pool(name="psum_y", bufs=4, space="PSUM"))

        wt_s = consts.tile([B, KB, N], BF16)
        ident = consts.tile([B, B], BF16)
        nc.sync.dma_start(ident[:], id_d.ap())
        nc.sync.dma_start(wt_s[:, 0:2], wt_r[:, 0:2])

        xts = {}
        yts = {}

        def get_xt(g):
            # per-m-tile loads: finer-grained prefetch, shorter pipeline head.
            # In group 0, only the first two tiles load up front; the rest
            # (and the 2 MB weight) are deferred behind the first reduce so
            # the m-tile-0 pipeline isn't starved by DMA round-robin.
            if g not in xts:
                xt = xin.tile([B, group, K], F32, tag="xt", name="xt")
                for j in range(2 if g == 0 else group):
                    nc.sync.dma_start(xt[:, j], x_r[g * group + j])
                xts[g] = xt
            return xts[g]

        def stage_a(t):
            """quant + transpose-dequant: produces xT (bf16 [k, m]) for tile t."""
            g, j = divmod(t, group)
            xmk = get_xt(g)[:, j]                        # [128, 1024] f32

            amax = work.tile([B, KB], F32, tag="amax", name="amax")
            red_inst = nc.vector.tensor_reduce(
                amax[:],
                xmk.rearrange("p (kb b) -> p kb b", b=B),
                axis=mybir.AxisListType.X,
                op=mybir.AluOpType.max,
                apply_absolute_value=True,
            )
            if t == 0:
                for jj in range(2, group):
                    dep = nc.sync.dma_start(get_xt(0)[:, jj], x_r[jj])
                    tile.add_dep_helper(
                        dep.ins, red_inst.ins, reason="defer load behind mtile0"
                    )
                # weight k-blocks 0-1 load up front (early GEMM matmuls need
                # only it); the remaining 7/8 of the weight follows behind
                # the first reduce so it doesn't starve the m-tile-0 chain.
                dep = nc.sync.dma_start(wt_s[:, 2:], wt_r[:, 2:])
                tile.add_dep_helper(
                    dep.ins, red_inst.ins, reason="defer w load behind mtile0"
                )
            # xs = max(amax, 448e-12) / 224  (one fused clamp+scale op)
            xs = work.tile([B, KB], F32, tag="xs", name="xs")
            nc.vector.tensor_scalar(
                xs[:], amax[:], 448e-12, 1.0 / FP8_HW_MAX,
                op0=mybir.AluOpType.max, op1=mybir.AluOpType.mult,
            )
            rxs = work.tile([B, KB], F32, tag="rxs", name="rxs")
            nc.vector.reciprocal(rxs[:], xs[:])

            # quantize: xq = fp8e4(x * 224/amax) -- one broadcast multiply
            xq = work.tile([B, K], FP8, tag="xq", name="xq")
            nc.vector.tensor_tensor(
                xq[:].rearrange("p (kb b) -> p kb b", b=B),
                xmk.rearrange("p (kb b) -> p kb b", b=B),
                rxs[:, :, None].to_broadcast((B, KB, B)),
                mybir.AluOpType.mult,
            )
            # diag(xs_kb) blocks = I * xs in one broadcast multiply on DVE;
            # keeping GpSimd idle avoids the DVE<->GpSimd SBUF-port lock
            # that would halve DVE throughput while GpSimd streams.
            diag8 = work.tile([B, KB, B], BF16, tag="diag8", name="diag8")
            nc.vector.tensor_tensor(
                diag8[:],
                ident[:, None, :].to_broadcast((B, KB, B)),
                xs[:, :, None].to_broadcast((B, KB, B)),
                mybir.AluOpType.mult,
            )

            # transpose + dequant: psum[k, m] = sum_m' xq[m',k] diag[m',m]
            pt0 = psum_t.tile([B, 512], F32, tag="pt", name="pt0")
            pt1 = psum_t.tile([B, 512], F32, tag="pt", name="pt1")
            for kb in range(KB):
                pt = pt0 if kb < 4 else pt1
                nc.tensor.matmul(
                    pt[:, (kb % 4) * B:(kb % 4 + 1) * B],
                    xq[:, kb * B:(kb + 1) * B],
                    diag8[:, kb],
                    start=True,
                    stop=True,
                )
            xT = work.tile([B, K], BF16, tag="xT", name="xT")
            nc.scalar.copy(xT[:, 0:512], pt0[:])
            nc.scalar.copy(xT[:, 512:1024], pt1[:])
            return xT

        def stage_b(t, xT):
            """main GEMM + output copy/DMA for tile t."""
            g, j = divmod(t, group)
            if g not in yts:
                yts[g] = yout.tile([B, group, N], F32, tag="yt", name="yt")
            yt = yts[g]
            py0 = psum_y.tile([B, 512], F32, tag="py", name="py0")
            py1 = psum_y.tile([B, 512], F32, tag="py", name="py1")
            for kb in range(KB):
                lhsT = xT[:, kb * B:(kb + 1) * B]
                nc.tensor.matmul(
                    py0[:], lhsT, wt_s[:, kb, 0:512],
                    start=(kb == 0), stop=(kb == KB - 1),
                )
                nc.tensor.matmul(
                    py1[:], lhsT, wt_s[:, kb, 512:1024],
                    start=(kb == 0), stop=(kb == KB - 1),
                )
            nc.scalar.copy(yt[:, j, 0:512], py0[:])
            nc.scalar.copy(yt[:, j, 512:1024], py1[:])
            nc.sync.dma_start(y_r[t], yt[:, j])

        # software pipeline (depth 2): transposes run up to two m-tiles
        # ahead of the GEMM, so the PE always has independent fill work
        # when a GEMM briefly waits on PSUM recycling or ACT copies.
        from collections import deque
        pending = deque()
        for t in range(mt):
            pending.append((t, stage_a(t)))
            if len(pending) > 2:
                stage_b(*pending.popleft())
        while pending:
            stage_b(*pending.popleft())

    nc.compile()
    nc.m = get_hw_module(nc.m)
    return nc


def host_prep(weight, w_scale):
    weight = np.asarray(weight)
    if weight.dtype != ml_dtypes.float8_e4m3fn:
        weight = weight.view(ml_dtypes.float8_e4m3fn)
    w_scale = np.asarray(w_scale, dtype=np.float32)
    nb, kb = w_scale.shape
    w_deq = (
        weight.astype(np.float32).reshape(nb, B, kb, B)
        * w_scale[:, None, :, None]
    ).reshape(nb * B, kb * B)
    wt = np.ascontiguousarray(w_deq.T).astype(ml_dtypes.bfloat16)  # [K, N]
    ident = np.eye(B, dtype=ml_dtypes.bfloat16)
    return wt, ident


_NC_CACHE = {}


def _get_nc(ms):
    if ms not in _NC_CACHE:
        _NC_CACHE[ms] = build_bass(ms)
    return _NC_CACHE[ms]


def kernel(x, weight, w_scale, _trace=False):
    x = np.ascontiguousarray(np.asarray(x, dtype=np.float32))
    assert x.shape == (M, K), x.shape
    wt, ident = host_prep(weight, w_scale)
    nc = _get_nc(MS)
    in_maps = [
        {"x": x[c * MS:(c + 1) * MS], "wt": wt, "ident": ident}
        for c in range(NCORES)
    ]
    res = run_bass_kernel_spmd(
        nc, in_maps, core_ids=list(range(NCORES)), trace=_trace
    )
    y = np.concatenate([r["y"] for r in res.results], axis=0)
    if _trace:
        return y, res
    return y


# revision 20
# speedup vs baseline: 1.3398x; 1.0052x over previous
"""BlockwiseQuantLinear Trainium2 kernel.

y = x_deq @ w_deq.T where
  x_deq = fp8-blockwise-quantize-dequantize(x)  (1x128 blocks along K)
  w_deq = fp8 weight * 128x128 blockwise scales

Strategy: data-parallel over M across the 8 NeuronCores (8192 rows each).
Weight is dequantized to bf16 on host (small: 1024x1024), transposed to
[K, N], and replicated. On device, per 128-row m-tile:
  1. DVE: segmented abs-max over 1x128 blocks -> per-(row,block) scales.
     TRN2's fp8e4 is IEEE e4m3 (max 240), not e4m3fn (max 448), so we
     quantize with 224/amax: identical RTNE rounding up to a power of two.
  2. DVE: x * (224/amax) -> xq in fp8e4, and diag matrices diag(amax/224).
  3. PE: xq_block^T @ diag -> PSUM, which both transposes the block (K on
     partitions) and applies the dequant scale in one matmul.
  4. ACT: PSUM -> SBUF bf16 copies (x_deq^T tiles).
  5. PE: bf16 GEMM, 8 k-block matmuls accumulating into PSUM per 512-col
     half; ACT copies PSUM -> SBUF fp32; batched DMA out.
"""

import numpy as np
import ml_dtypes
from contextlib import ExitStack

import concourse.bacc as bacc
import concourse.mybir as mybir
import concourse.tile as tile
from concourse.bass_utils import run_bass_kernel_spmd
from concourse.bass_interp import get_hw_module

M, K, N = 65536, 1024, 1024
NCORES = 8
MS = M // NCORES          # 8192 rows per core
B = 128                   # quant block size
KB = K // B               # 8 k-blocks
NB = N // B
GROUP = 4                 # m-tiles per DMA batch (2 MB transfers)
FP8_HW_MAX = 224.0        # trn2 fp8e4 is IEEE e4m3 (max 240); 224 = 448/2

F32 = mybir.dt.float32
BF16 = mybir.dt.bfloat16
FP8 = mybir.dt.float8e4


def build_bass(ms: int = MS, group: int = GROUP):
    """Build + compile the per-core Bass program for an ms-row shard."""
    mt = ms // B                      # m-tiles
    ngroups = mt // group
    assert mt % group == 0

    nc = bacc.Bacc(
        "TRN2", target_bir_lowering=False, debug=False, num_devices=NCORES
    )
    x_d = nc.dram_tensor("x", [ms, K], F32, kind="ExternalInput")
    wt_d = nc.dram_tensor("wt", [K, N], BF16, kind="ExternalInput")
    id_d = nc.dram_tensor("ident", [B, B], BF16, kind="ExternalInput")
    y_d = nc.dram_tensor("y", [ms, N], F32, kind="ExternalOutput")

    x_r = x_d.ap().rearrange("(t p) k -> t p k", p=B)    # [mt, 128, 1024]
    y_r = y_d.ap().rearrange("(t p) n -> t p n", p=B)
    wt_r = wt_d.ap().rearrange("(kb p) n -> p kb n", p=B)

    with tile.TileContext(nc) as tc, ExitStack() as ctx:
        consts = ctx.enter_context(tc.tile_pool(name="consts", bufs=1))
        xin = ctx.enter_context(tc.tile_pool(name="xin", bufs=3))
        yout = ctx.enter_context(tc.tile_pool(name="yout", bufs=3))
        work = ctx.enter_context(tc.tile_pool(name="work", bufs=6))
        psum_t = ctx.enter_context(tc.tile_pool(name="psum_t", bufs=4, space="PSUM"))
        psum_y = ctx.enter_context(tc.tile_pool(name="psum_y", bufs=4, space="PSUM"))

        wt_s = consts.tile([B, KB, N], BF16)
        ident = consts.tile([B, B], BF16)
        nc.sync.dma_start(ident[:], id_d.ap())

        xts = {}
        yts = {}

        def get_xt(g):
            # per-m-tile loads: finer-grained prefetch, shorter pipeline head.
            # In group 0, m-tile 0 loads in two k-halves ahead of everything
            # else (so its quant chain starts ASAP), the first weight blocks
            # follow, and the rest is deferred behind the first reduce so the
            # m-tile-0 pipeline isn't starved by DMA round-robin.
            if g not in xts:
                xt = xin.tile([B, group, K], F32, tag="xt", name="xt")
                if g == 0:
                    nc.sync.dma_start(xt[:, 0, 0:K // 2], x_r[0][:, 0:K // 2])
                    nc.sync.dma_start(xt[:, 0, K // 2:], x_r[0][:, K // 2:])
                    nc.sync.dma_start(xt[:, 1], x_r[1])
                    nc.sync.dma_start(wt_s[:, 0:2], wt_r[:, 0:2])
                else:
                    for j in range(group):
                        nc.sync.dma_start(xt[:, j], x_r[g * group + j])
                xts[g] = xt
            return xts[g]

        def stage_a(t):
            """quant + transpose-dequant: produces xT (bf16 [k, m]) for tile t.

            m-tile 0 runs the chain in two k-halves so compute starts as
            soon as the first 256 KB of x lands; steady-state tiles use
            single full-width ops (less per-op overhead).
            """
            g, j = divmod(t, group)
            xmk = get_xt(g)[:, j]                        # [128, 1024] f32
            halves = 2 if t == 0 else 1
            kbh = KB // halves

            amax = work.tile([B, KB], F32, tag="amax", name="amax")
            xs = work.tile([B, KB], F32, tag="xs", name="xs")
            rxs = work.tile([B, KB], F32, tag="rxs", name="rxs")
            xq = work.tile([B, K], FP8, tag="xq", name="xq")
            diag8 = work.tile([B, KB, B], BF16, tag="diag8", name="diag8")
            pt0 = psum_t.tile([B, 512], F32, tag="pt", name="pt0")
            pt1 = psum_t.tile([B, 512], F32, tag="pt", name="pt1")
            xT = work.tile([B, K], BF16, tag="xT", name="xT")

            for h in range(halves):
                kbs = slice(h * kbh, (h + 1) * kbh)
                ksl = slice(h * kbh * B, (h + 1) * kbh * B)
                red_inst = nc.vector.tensor_reduce(
                    amax[:, kbs],
                    xmk[:, ksl].rearrange("p (kb b) -> p kb b", b=B),
                    axis=mybir.AxisListType.X,
                    op=mybir.AluOpType.max,
                    apply_absolute_value=True,
                )
                if t == 0 and h == 0:
                    for jj in range(2, group):
                        dep = nc.sync.dma_start(get_xt(0)[:, jj], x_r[jj])
                        tile.add_dep_helper(
                            dep.ins, red_inst.ins,
                            reason="defer load behind mtile0",
                        )
                    # weight k-blocks 0-1 load up front (early GEMM matmuls
                    # need only them); the remaining 6/8 of the weight
                    # follows behind the first reduce so it doesn't starve
                    # the m-tile-0 chain.
                    dep = nc.sync.dma_start(wt_s[:, 2:], wt_r[:, 2:])
                    tile.add_dep_helper(
                        dep.ins, red_inst.ins,
                        reason="defer w load behind mtile0",
                    )
                # xs = max(amax, 448e-12) / 224  (one fused clamp+scale op)
                nc.vector.tensor_scalar(
                    xs[:, kbs], amax[:, kbs], 448e-12, 1.0 / FP8_HW_MAX,
                    op0=mybir.AluOpType.max, op1=mybir.AluOpType.mult,
                )
                nc.vector.reciprocal(rxs[:, kbs], xs[:, kbs])

                # quantize: xq = fp8e4(x * 224/amax) -- one broadcast multiply
                nc.vector.tensor_tensor(
                    xq[:, ksl].rearrange("p (kb b) -> p kb b", b=B),
                    xmk[:, ksl].rearrange("p (kb b) -> p kb b", b=B),
                    rxs[:, kbs, None].to_broadcast((B, kbh, B)),
                    mybir.AluOpType.mult,
                )
                # diag(xs_kb) blocks = I * xs in one broadcast multiply on
                # DVE; keeping GpSimd idle avoids the DVE<->GpSimd SBUF-port
                # lock that would halve DVE throughput while GpSimd streams.
                nc.vector.tensor_tensor(
                    diag8[:, kbs],
                    ident[:, None, :].to_broadcast((B, kbh, B)),
                    xs[:, kbs, None].to_broadcast((B, kbh, B)),
                    mybir.AluOpType.mult,
                )

                # transpose + dequant: psum[k,m] = sum_m' xq[m',k] diag[m',m]
                for kb in range(h * kbh, (h + 1) * kbh):
                    pt = pt0 if kb < 4 else pt1
                    nc.tensor.matmul(
                        pt[:, (kb % 4) * B:(kb % 4 + 1) * B],
                        xq[:, kb * B:(kb + 1) * B],
                        diag8[:, kb],
                        start=True,
                        stop=True,
                    )
                if halves == 2:
                    # half-aligned PSUM->SBUF copy right away
                    nc.scalar.copy(xT[:, h * 512:(h + 1) * 512],
                                   (pt0 if h == 0 else pt1)[:])
            if halves == 1:
                nc.scalar.copy(xT[:, 0:512], pt0[:])
                nc.scalar.copy(xT[:, 512:1024], pt1[:])
            return xT

        def stage_b(t, xT):
            """main GEMM + output copy/DMA for tile t."""
            g, j = divmod(t, group)
            if g not in yts:
                yts[g] = yout.tile([B, group, N], F32, tag="yt", name="yt")
            yt = yts[g]
            py0 = psum_y.tile([B, 512], F32, tag="py", name="py0")
            py1 = psum_y.tile([B, 512], F32, tag="py", name="py1")
            for kb in range(KB):
                lhsT = xT[:, kb * B:(kb + 1) * B]
                nc.tensor.matmul(
                    py0[:], lhsT, wt_s[:, kb, 0:512],
                    start=(kb == 0), stop=(kb == KB - 1),
                )
                nc.tensor.matmul(
                    py1[:], lhsT, wt_s[:, kb, 512:1024],
                    start=(kb == 0), stop=(kb == KB - 1),
                )
            nc.scalar.copy(yt[:, j, 0:512], py0[:])
            if t == mt - 1:
                # last tile: ship each half as soon as its copy lands so the
                # kernel tail only waits on a 256 KB transfer
                nc.sync.dma_start(y_r[t][:, 0:512], yt[:, j, 0:512])
                nc.scalar.copy(yt[:, j, 512:1024], py1[:])
                nc.sync.dma_start(y_r[t][:, 512:1024], yt[:, j, 512:1024])
            else:
                nc.scalar.copy(yt[:, j, 512:1024], py1[:])
                nc.sync.dma_start(y_r[t], yt[:, j])

        # software pipeline (depth 2): transposes run up to two m-tiles
        # ahead of the GEMM, so the PE always has independent fill work
        # when a GEMM briefly waits on PSUM recycling or ACT copies.
        from collections import deque
        pending = deque()
        for t in range(mt):
            pending.append((t, stage_a(t)))
            if len(pending) > 2:
                stage_b(*pending.popleft())
        while pending:
            stage_b(*pending.popleft())

    nc.compile()
    nc.m = get_hw_module(nc.m)
    return nc


def host_prep(weight, w_scale):
    weight = np.asarray(weight)
    if weight.dtype != ml_dtypes.float8_e4m3fn:
        weight = weight.view(ml_dtypes.float8_e4m3fn)
    w_scale = np.asarray(w_scale, dtype=np.float32)
    nb, kb = w_scale.shape
    w_deq = (
        weight.astype(np.float32).reshape(nb, B, kb, B)
        * w_scale[:, None, :, None]
    ).reshape(nb * B, kb * B)
    wt = np.ascontiguousarray(w_deq.T).astype(ml_dtypes.bfloat16)  # [K, N]
    ident = np.eye(B, dtype=ml_dtypes.bfloat16)
    return wt, ident


_NC_CACHE = {}


def _get_nc(ms):
    if ms not in _NC_CACHE:
        _NC_CACHE[ms] = build_bass(ms)
    return _NC_CACHE[ms]


def kernel(x, weight, w_scale, _trace=False):
    x = np.ascontiguousarray(np.asarray(x, dtype=np.float32))
    assert x.shape == (M, K), x.shape
    wt, ident = host_prep(weight, w_scale)
    nc = _get_nc(MS)
    in_maps = [
        {"x": x[c * MS:(c + 1) * MS], "wt": wt, "ident": ident}
        for c in range(NCORES)
    ]
    res = run_bass_kernel_spmd(
        nc, in_maps, core_ids=list(range(NCORES)), trace=_trace
    )
    y = np.concatenate([r["y"] for r in res.results], axis=0)
    if _trace:
        return y, res
    return y
